# revision 1
# baseline (speedup 1.0000x reference)
"""Causal single-head attention (B=4, T=2048, E=1024, H=128) on 8 NeuronCores.

Sharding: core = (batch b, half h). Each core computes the attention output for
one half (1024 rows) of one batch. Per-core input xt is x[b]^T with the key
column-blocks permuted so the core's OWN query half always sits at columns
1024:2048 (so the SPMD program is identical across cores); causality for the
permuted key order is enforced via per-core additive mask DATA, not code.

Pipeline per core (all on-chip, one launch):
  xT (f32r) --PE--> qT,kT,vT [H,T] (f32r)   (W stationary, x moving, E chunked)
  vT --PE transpose--> v natural [T,H] (fp16, + ones column for denominators)
  scores^T [keys,q] = kT_blk^T @ qT_grp (f32r, N=256)  + additive causal masks
  expS = exp(scores^T) (ACT, fp16)
  out[q,H] | denom[q,1] = sum_kb expS_blk^T @ [v|1]  (fp16 matmul, fp32 PSUM)
  y = out * (1/denom)  (DVE)
Softmax skips max-subtraction: scores ~ N(0,1), exp is safe in fp32/fp16 range.
"""

import math

import numpy as np

import concourse.bass as bass
import concourse.tile as tile
from concourse import bacc, mybir
from concourse.bass_utils import run_bass_kernel_spmd
from concourse.masks import make_identity

B, T, E, H = 4, 2048, 1024, 128
NB = T // 128          # 16 key slots
NE = E // 128          # 8 contraction chunks
NG = 4                 # q groups of 256 (= 8 local q blocks)
BIG = 1.0e30

F32 = mybir.dt.float32
F32R = mybir.dt.float32r
F16 = mybir.dt.float16

_CACHE: dict = {}


def _build():
    nc = bacc.Bacc(None, target_bir_lowering=False)
    xt = nc.dram_tensor("xt", [E, T], F32R, kind="ExternalInput")
    wq = nc.dram_tensor("wq", [E, H], F32R, kind="ExternalInput")
    wk = nc.dram_tensor("wk", [E, H], F32R, kind="ExternalInput")
    wv = nc.dram_tensor("wv", [E, H], F32R, kind="ExternalInput")
    bq = nc.dram_tensor("bq", [H, 1], F32, kind="ExternalInput")
    bk = nc.dram_tensor("bk", [H, 1], F32, kind="ExternalInput")
    msk = nc.dram_tensor("msk", [128, 2304], F32, kind="ExternalInput")
    y = nc.dram_tensor("y", [T // 2, H], F32, kind="ExternalOutput")

    with tile.TileContext(nc) as tc:
        with (
            tc.tile_pool(name="xs", bufs=1) as xs_pool,
            tc.tile_pool(name="wsb", bufs=1) as w_pool,
            tc.tile_pool(name="small", bufs=1) as sm_pool,
            tc.tile_pool(name="qk", bufs=1) as qk_pool,
            tc.tile_pool(name="ex", bufs=8) as ex_pool,
            tc.tile_pool(name="ob", bufs=3) as ob_pool,
        ):
            xs = xs_pool.tile([128, NE * T], F32R)
            for e in range(NE):
                for qr in range(4):
                    eng = nc.gpsimd if (e * 4 + qr) % 2 == 0 else nc.sync
                    eng.dma_start(
                        xs[:, e * T + qr * (T // 4): e * T + (qr + 1) * (T // 4)],
                        xt[e * 128:(e + 1) * 128, qr * (T // 4):(qr + 1) * (T // 4)])
            wsb = w_pool.tile([128, 3 * E], F32R)
            for p, w in enumerate((wq, wk, wv)):
                for e in range(NE):
                    nc.sync.dma_start(
                        wsb[:, p * E + e * 128: p * E + (e + 1) * 128],
                        w[e * 128:(e + 1) * 128, :])
            bq_sb = sm_pool.tile([128, 1], F32, tag="bq")
            bk_sb = sm_pool.tile([128, 1], F32, tag="bk")
            nc.sync.dma_start(bq_sb[:], bq[:])
            nc.sync.dma_start(bk_sb[:], bk[:])
            mask_sb = sm_pool.tile([128, 2304], F32, tag="msk")
            nc.sync.dma_start(mask_sb[:], msk[:])
            ident = sm_pool.tile([128, 128], F32, tag="ident")
            make_identity(nc, ident[:])

            qT = qk_pool.tile([128, T // 2], F32R, tag="qT")
            kTs = [qk_pool.tile([128, 512], F32R, name=f"kT{t}", tag=f"kT{t}") for t in range(4)]
            vTs = [qk_pool.tile([128, 512], F32, name=f"vT{t}", tag=f"vT{t}") for t in range(4)]
            vaugs = [qk_pool.tile([128, 4 * 129], F16, name=f"vaug{t}", tag=f"vaug{t}") for t in range(4)]
            for t in range(4):
                nc.vector.memset(vaugs[t][:], 1.0)

            def kT(kb):
                return kTs[kb // 4][:, (kb % 4) * 128:(kb % 4 + 1) * 128]

            def vaug(kb):
                return vaugs[kb // 4][:, (kb % 4) * 129:(kb % 4 + 1) * 129]

            # ---- projections + v transposes, interleaved per 512-slot so
            # attention groups unblock as early as possible
            with (
                tc.tile_pool(name="pps", bufs=3, space="PSUM") as pps,
                tc.tile_pool(name="tps", bufs=2, space="PSUM") as tps,
            ):
                # qT first: every attention group needs it
                for t in range(2):
                    ps = pps.tile([128, 512], F32)
                    for e in range(NE):
                        nc.tensor.matmul(
                            ps[:],
                            wsb[:, e * 128:(e + 1) * 128],
                            xs[:, e * T + 1024 + t * 512: e * T + 1024 + (t + 1) * 512],
                            start=(e == 0), stop=(e == NE - 1))
                    nc.scalar.activation(
                        qT[:, t * 512:(t + 1) * 512], ps[:],
                        mybir.ActivationFunctionType.Identity, bias=bq_sb[:])
                for t in range(4):
                    ps = pps.tile([128, 512], F32)
                    for e in range(NE):
                        nc.tensor.matmul(
                            ps[:],
                            wsb[:, E + e * 128: E + (e + 1) * 128],
                            xs[:, e * T + t * 512: e * T + (t + 1) * 512],
                            start=(e == 0), stop=(e == NE - 1))
                    nc.scalar.activation(
                        kTs[t][:], ps[:],
                        mybir.ActivationFunctionType.Identity, bias=bk_sb[:])
                    ps = pps.tile([128, 512], F32)
                    for e in range(NE):
                        nc.tensor.matmul(
                            ps[:],
                            wsb[:, 2 * E + e * 128: 2 * E + (e + 1) * 128],
                            xs[:, e * T + t * 512: e * T + (t + 1) * 512],
                            start=(e == 0), stop=(e == NE - 1))
                    nc.vector.tensor_copy(vTs[t][:], ps[:])
                    for s in range(4):
                        tp = tps.tile([128, 128], F32)
                        nc.tensor.transpose(tp[:], vTs[t][:, s * 128:(s + 1) * 128], ident[:])
                        nc.vector.tensor_copy(
                            vaugs[t][:, s * 129: s * 129 + 128], tp[:])

            # ---- attention: group j covers q cols [256j, 256j+256)
            with (
                tc.tile_pool(name="sps", bufs=3, space="PSUM") as sps,
                tc.tile_pool(name="avps", bufs=2, space="PSUM") as avps,
            ):
                for j in range(2):
                    kj = 12 + 4 * j
                    av = [avps.tile([128, 129], F32, name=f"av{q}", tag=f"av{q}", bufs=1) for q in range(4)]
                    for kb in range(kj):
                        sp = sps.tile([128, 512], F32)
                        nc.tensor.matmul(
                            sp[:], kT(kb),
                            qT[:, j * 512:(j + 1) * 512], start=True, stop=True)
                        ebias = 0.0
                        if kb < 8:
                            # M_low is per-partition constant: fold into exp bias
                            ebias = mask_sb[:, 0:1]
                        elif kb >= kj - 4:
                            r = kb - (kj - 4)
                            nc.vector.tensor_add(
                                sp[:], sp[:], mask_sb[:, 256 + r * 512: 256 + (r + 1) * 512])
                        ex = ex_pool.tile([128, 512], F16)
                        nc.scalar.activation(ex[:], sp[:], mybir.ActivationFunctionType.Exp,
                                             bias=ebias)
                        for q in range(4):
                            nc.tensor.matmul(
                                av[q][:],
                                ex[:, q * 128:(q + 1) * 128],
                                vaug(kb),
                                start=(kb == 0), stop=(kb == kj - 1))
                    for q in range(4):
                        rc = ob_pool.tile([128, 1], F32, tag="rc")
                        nc.vector.reciprocal(rc[:], av[q][:, 128:129])
                        ob = ob_pool.tile([128, 128], F32, tag="ob")
                        nc.vector.tensor_scalar_mul(ob[:], av[q][:, 0:128], rc[:])
                        nc.scalar.dma_start(y[(4 * j + q) * 128:(4 * j + q + 1) * 128, :], ob[:])
    nc.compile()
    return nc


def _masks(h: int) -> np.ndarray:
    p = np.arange(128)[:, None]  # key within block (partition)
    c = np.arange(128)[None, :]  # query within block (free)
    tri = np.where(p <= c, 0.0, -BIG).astype(np.float32)
    z = np.zeros((128, 128), np.float32)
    inv = np.full((128, 128), -BIG, np.float32)
    m_low = (z if h == 1 else inv)[:, :128]
    parts = [np.concatenate([m_low, m_low], axis=1)]  # cols 0:256 (only col 0 used)
    for r in range(4):  # window mask W_r for the last 4 kbs of a 512-q group
        quarters = [z if r < cq else (tri if r == cq else inv) for cq in range(4)]
        parts.append(np.concatenate(quarters, axis=1))
    return np.ascontiguousarray(np.concatenate(parts, axis=1))


def kernel(x, Wq, bq, Wk, bk, Wv, bv):
    x = np.asarray(x, dtype=np.float32)
    Wq = np.asarray(Wq, dtype=np.float32)
    Wk = np.asarray(Wk, dtype=np.float32)
    Wv = np.asarray(Wv, dtype=np.float32)
    bq = np.asarray(bq, dtype=np.float32)
    bk = np.asarray(bk, dtype=np.float32)
    bv = np.asarray(bv, dtype=np.float32)

    if "nc" not in _CACHE:
        _CACHE["nc"] = _build()
    nc = _CACHE["nc"]

    scale = 1.0 / math.sqrt(H)
    wq_s = np.ascontiguousarray(Wq * scale)
    bq_s = np.ascontiguousarray((bq * scale).reshape(H, 1))
    bk_r = np.ascontiguousarray(bk.reshape(H, 1))
    masks = {0: _masks(0), 1: _masks(1)}

    xt = np.ascontiguousarray(x.transpose(0, 2, 1))  # [B, E, T]
    in_maps = []
    for core in range(8):
        b, h = divmod(core, 2)
        if h == 1:
            xtc = xt[b]
        else:
            xtc = np.ascontiguousarray(
                np.concatenate([xt[b][:, T // 2:], xt[b][:, :T // 2]], axis=1))
        in_maps.append({
            "xt": xtc, "wq": wq_s, "wk": Wk, "wv": Wv,
            "bq": bq_s, "bk": bk_r, "msk": masks[h],
        })

    res = run_bass_kernel_spmd(nc, in_maps, core_ids=list(range(8)))
    out = np.empty((B, T, H), dtype=np.float32)
    for core in range(8):
        b, h = divmod(core, 2)
        out[b, h * (T // 2):(h + 1) * (T // 2), :] = res.results[core]["y"]
    out += bv  # sum_j softmax_ij = 1, so +bv commutes with attention
    return out



# revision 7
# speedup vs baseline: 1.2570x; 1.2570x over previous
"""Causal single-head attention (B=4, T=2048, E=1024, H=128) on 8 NeuronCores.

Sharding: core = (batch b, half c). Both cores of a batch compute q for ALL
T=2048 queries; the KEYS are split between them in interleaved block pairs
(core 0: key blocks {0,1,4,5,8,9,12,13}, core 1: {2,3,6,7,10,11,14,15}).
For query group g (512 queries = blocks 4g..4g+3), key slots 0..2g+1 of each
core exactly cover the causally needed key range [0, 4g+4) disjointly across
the pair, so the per-core work is perfectly balanced and the SPMD program is
identical — only the key-column data and the two window masks differ per core.

Each core produces PARTIAL attention sums: num[H, T] = sum_k exp(s)·v and
den[1, T] = sum_k exp(s); the host combines halves and normalizes:
out = ((num0+num1)/(den0+den1))^T + bv.

Pipeline per core (fp16 matmul inputs, fp32 PSUM accumulation):
  k,v proj: W stationary, gathered own-key x columns moving  -> kT, vT [H,1024]
  vT --PE transpose--> v natural [keys,H] per slot (fp16)
  q proj: W stationary, full x moving -> qT [H, 2048] (scale folded into Wq)
  per group g: scores^T[keys,512q] = kT_s^T @ qT_g; window masks on the last
  two slots; exp (ACT, fp16); AV: v_s stationary, ex moving -> num^T accum;
  den: DVE slot-sum of ex then one ones-row matmul.
Softmax skips max-subtraction: scores ~ N(0,1), exp is safe in fp16 range.
"""

import math

import numpy as np

import concourse.bass as bass
import concourse.tile as tile
from concourse import bacc, mybir
from concourse.bass_utils import run_bass_kernel_spmd
from concourse.masks import make_identity

B, T, E, H = 4, 2048, 1024, 128
NE = E // 128           # 8 contraction chunks
NS = 8                  # own key slots (128 keys each) per core
KV = NS * 128           # 1024 own key columns
BIG = 1.0e30

F32 = mybir.dt.float32
F16 = mybir.dt.float16

_CACHE: dict = {}


def _build():
    nc = bacc.Bacc(None, target_bir_lowering=False)
    xt = nc.dram_tensor("xt", [E, T], F16, kind="ExternalInput")
    xkv = nc.dram_tensor("xkv", [E, KV], F16, kind="ExternalInput")
    wq = nc.dram_tensor("wq", [E, H], F16, kind="ExternalInput")
    wk = nc.dram_tensor("wk", [E, H], F16, kind="ExternalInput")
    wv = nc.dram_tensor("wv", [E, H], F16, kind="ExternalInput")
    bq = nc.dram_tensor("bq", [H, 1], F32, kind="ExternalInput")
    bk = nc.dram_tensor("bk", [H, 1], F32, kind="ExternalInput")
    msk = nc.dram_tensor("msk", [128, 1024], F32, kind="ExternalInput")
    y_num = nc.dram_tensor("y_num", [H, T], F16, kind="ExternalOutput")
    y_den = nc.dram_tensor("y_den", [1, T], F32, kind="ExternalOutput")

    with tile.TileContext(nc) as tc:
        with (
            tc.tile_pool(name="xs", bufs=1) as xs_pool,
            tc.tile_pool(name="wsb", bufs=1) as w_pool,
            tc.tile_pool(name="small", bufs=1) as sm_pool,
            tc.tile_pool(name="qk", bufs=1) as qk_pool,
            tc.tile_pool(name="ex", bufs=10) as ex_pool,
        ):
            # ---- DMA in: weights first, then own-key columns, then x quarters
            wsb = w_pool.tile([128, 3 * E], F16)
            for p, w in enumerate((wk, wv, wq)):
                for e in range(NE):
                    nc.scalar.dma_start(
                        wsb[:, p * E + e * 128: p * E + (e + 1) * 128],
                        w[e * 128:(e + 1) * 128, :])
            bq_sb = sm_pool.tile([128, 1], F32, tag="bq")
            bk_sb = sm_pool.tile([128, 1], F32, tag="bk")
            nc.scalar.dma_start(bq_sb[:], bq[:])
            nc.scalar.dma_start(bk_sb[:], bk[:])
            mask_sb = sm_pool.tile([128, 1024], F32, tag="msk")
            nc.scalar.dma_start(mask_sb[:], msk[:])
            ident = sm_pool.tile([128, 128], F16, tag="ident")
            make_identity(nc, ident[:])
            ones = sm_pool.tile([128, 1], F16, tag="ones")
            nc.vector.memset(ones[:], 1.0)

            xkv_sb = xs_pool.tile([128, NE * KV], F16, name="xkv_sb", tag="xkv")
            for e in range(NE):
                eng = nc.gpsimd if e % 2 == 0 else nc.sync
                eng.dma_start(
                    xkv_sb[:, e * KV:(e + 1) * KV],
                    xkv[e * 128:(e + 1) * 128, :])
            xs = xs_pool.tile([128, NE * T], F16, name="xs", tag="xs")
            for qr in range(4):
                for e in range(NE):
                    eng = nc.gpsimd if (qr * NE + e) % 2 == 0 else nc.sync
                    eng.dma_start(
                        xs[:, e * T + qr * 512: e * T + (qr + 1) * 512],
                        xt[e * 128:(e + 1) * 128, qr * 512:(qr + 1) * 512])

            kT = qk_pool.tile([128, KV], F16, tag="kT")
            vT = qk_pool.tile([128, KV], F16, tag="vT")
            vnat = qk_pool.tile([128, NS * 128], F16, tag="vnat")
            qT = qk_pool.tile([128, T], F16, tag="qT")
            sum_ex = qk_pool.tile([128, 512], F16, tag="sum_ex")

            # ---- k,v projections (own key columns), W stationary per e-chunk
            with tc.tile_pool(name="kvps", bufs=1, space="PSUM") as kvps:
                kps = [kvps.tile([128, 512], F32, name=f"kps{i}", tag=f"kps{i}") for i in range(2)]
                vps = [kvps.tile([128, 512], F32, name=f"vps{i}", tag=f"vps{i}") for i in range(2)]
                for e in range(NE):
                    st, sp_ = (e == 0), (e == NE - 1)
                    for i in range(2):
                        nc.tensor.matmul(
                            kps[i][:], wsb[:, e * 128:(e + 1) * 128],
                            xkv_sb[:, e * KV + i * 512: e * KV + (i + 1) * 512],
                            start=st, stop=sp_)
                    for i in range(2):
                        nc.tensor.matmul(
                            vps[i][:], wsb[:, E + e * 128: E + (e + 1) * 128],
                            xkv_sb[:, e * KV + i * 512: e * KV + (i + 1) * 512],
                            start=st, stop=sp_)
                for i in range(2):
                    nc.scalar.activation(
                        kT[:, i * 512:(i + 1) * 512], kps[i][:],
                        mybir.ActivationFunctionType.Identity, bias=bk_sb[:])
                    nc.vector.tensor_copy(vT[:, i * 512:(i + 1) * 512], vps[i][:])

            # ---- v transposes to natural [keys, H] per slot
            with tc.tile_pool(name="tps", bufs=2, space="PSUM") as tps:
                for s in range(NS):
                    tp = tps.tile([128, 128], F16)
                    nc.tensor.transpose(
                        tp[:], vT[:, s * 128:(s + 1) * 128], ident[:])
                    nc.vector.tensor_copy(vnat[:, s * 128:(s + 1) * 128], tp[:])

            # ---- q projections (2 waves of 2 groups) + attention per group
            with (
                tc.tile_pool(name="qps", bufs=1, space="PSUM") as qps_pool,
                tc.tile_pool(name="sps", bufs=3, space="PSUM") as sps,
                tc.tile_pool(name="aps", bufs=2, space="PSUM") as aps,
                tc.tile_pool(name="dps", bufs=1, space="PSUM") as dps,
            ):
                def q_wave(w):
                    qp = [qps_pool.tile([128, 512], F32, name=f"qp{w}{g}", tag=f"qp{g % 2}")
                          for g in range(2)]
                    for e in range(NE):
                        for g in range(2):
                            nc.tensor.matmul(
                                qp[g][:],
                                wsb[:, 2 * E + e * 128: 2 * E + (e + 1) * 128],
                                xs[:, e * T + (2 * w + g) * 512: e * T + (2 * w + g + 1) * 512],
                                start=(e == 0), stop=(e == NE - 1))
                    for g in range(2):
                        nc.scalar.activation(
                            qT[:, (2 * w + g) * 512:(2 * w + g + 1) * 512], qp[g][:],
                            mybir.ActivationFunctionType.Identity, bias=bq_sb[:])

                def attend(g):
                    L = 2 * g + 2
                    av = aps.tile([128, 512], F32, name=f"av{g}", tag="av")
                    den = dps.tile([1, 512], F32, name=f"den{g}", tag="den")
                    exs = []
                    for s in range(L):
                        sp = sps.tile([128, 512], F32)
                        nc.tensor.matmul(
                            sp[:], kT[:, s * 128:(s + 1) * 128],
                            qT[:, g * 512:(g + 1) * 512], start=True, stop=True)
                        if s >= L - 2:
                            j = s - (L - 2)
                            nc.vector.tensor_add(
                                sp[:], sp[:], mask_sb[:, j * 512:(j + 1) * 512])
                        ex = ex_pool.tile([128, 512], F16)
                        nc.scalar.activation(
                            ex[:], sp[:], mybir.ActivationFunctionType.Exp)
                        exs.append(ex)
                        nc.tensor.matmul(
                            av[:], vnat[:, s * 128:(s + 1) * 128], ex[:],
                            start=(s == 0), stop=(s == L - 1))
                        if s == 1:
                            nc.vector.tensor_add(sum_ex[:], exs[0][:], exs[1][:])
                        elif s > 1:
                            nc.vector.tensor_add(sum_ex[:], sum_ex[:], ex[:])
                    nc.tensor.matmul(den[:], ones[:], sum_ex[:], start=True, stop=True)
                    av_sb = ex_pool.tile([128, 512], F16, name=f"avsb{g}", tag="avsb")
                    nc.vector.tensor_copy(av_sb[:], av[:])
                    den_sb = ex_pool.tile([1, 512], F32, name=f"densb{g}", tag="densb")
                    nc.vector.tensor_copy(den_sb[:], den[:])
                    nc.scalar.dma_start(y_num[:, g * 512:(g + 1) * 512], av_sb[:])
                    nc.scalar.dma_start(y_den[:, g * 512:(g + 1) * 512], den_sb[:])

                q_wave(0)
                attend(0)
                q_wave(1)
                attend(1)
                attend(2)
                attend(3)
    nc.compile()
    return nc


def _masks(c: int) -> np.ndarray:
    p = np.arange(128)[:, None]  # key within block (partition)
    q = np.arange(128)[None, :]  # query within block (free)
    tri = np.where(p <= q, 0.0, -BIG).astype(np.float32)
    z = np.zeros((128, 128), np.float32)
    inv = np.full((128, 128), -BIG, np.float32)
    parts = []
    for r in (2 * c, 2 * c + 1):  # window mask W_r for key block 4g+r
        quarters = [z if r < cq else (tri if r == cq else inv) for cq in range(4)]
        parts.append(np.concatenate(quarters, axis=1))
    return np.ascontiguousarray(np.concatenate(parts, axis=1))


def _kv_cols(c: int) -> np.ndarray:
    # key blocks {4j + 2c, 4j + 2c + 1 : j in 0..3}, in increasing order
    blocks = []
    for j in range(4):
        blocks += [4 * j + 2 * c, 4 * j + 2 * c + 1]
    return np.concatenate([np.arange(b * 128, (b + 1) * 128) for b in blocks])


def kernel(x, Wq, bq, Wk, bk, Wv, bv):
    x = np.asarray(x, dtype=np.float32)
    Wq = np.asarray(Wq, dtype=np.float32)
    Wk = np.asarray(Wk, dtype=np.float32)
    Wv = np.asarray(Wv, dtype=np.float32)
    bq = np.asarray(bq, dtype=np.float32)
    bk = np.asarray(bk, dtype=np.float32)
    bv = np.asarray(bv, dtype=np.float32)

    if "nc" not in _CACHE:
        _CACHE["nc"] = _build()
    nc = _CACHE["nc"]

    scale = 1.0 / math.sqrt(H)
    wq_s = np.ascontiguousarray((Wq * scale).astype(np.float16))
    wk_h = np.ascontiguousarray(Wk.astype(np.float16))
    wv_h = np.ascontiguousarray(Wv.astype(np.float16))
    bq_s = np.ascontiguousarray((bq * scale).reshape(H, 1))
    bk_r = np.ascontiguousarray(bk.reshape(H, 1))
    masks = {0: _masks(0), 1: _masks(1)}
    cols = {0: _kv_cols(0), 1: _kv_cols(1)}

    xt = np.ascontiguousarray(x.transpose(0, 2, 1).astype(np.float16))  # [B,E,T]
    in_maps = []
    for core in range(8):
        b, c = divmod(core, 2)
        in_maps.append({
            "xt": xt[b],
            "xkv": np.ascontiguousarray(xt[b][:, cols[c]]),
            "wq": wq_s, "wk": wk_h, "wv": wv_h,
            "bq": bq_s, "bk": bk_r, "msk": masks[c],
        })

    res = run_bass_kernel_spmd(nc, in_maps, core_ids=list(range(8)))
    out = np.empty((B, T, H), dtype=np.float32)
    for b in range(B):
        r0, r1 = res.results[2 * b], res.results[2 * b + 1]
        num = r0["y_num"].astype(np.float32) + r1["y_num"].astype(np.float32)  # [H, T]
        den = r0["y_den"] + r1["y_den"]          # [1, T]
        out[b] = (num / den).T
    out += bv  # sum_j softmax_ij = 1, so +bv commutes with attention
    return out


# revision 13
# speedup vs baseline: 1.7896x; 1.4237x over previous
"""Causal single-head attention (B=4, T=2048, E=1024, H=128) on 8 NeuronCores.

Sharding: core = (batch b, half c). Both cores of a batch compute q for ALL
T=2048 queries; the KEYS are split between them in interleaved block pairs
(core 0: key blocks {0,1,4,5,8,9,12,13}, core 1: {2,3,6,7,10,11,14,15}).
For query group g (512 queries), key slots 0..2g+1 of each core exactly cover
the causally needed key range [0, 4g+4) disjointly across the pair, so the
per-core work is perfectly balanced and the SPMD program is identical.

To keep the program identical, the host swaps the two 256-column halves of
every 512-column quarter of x^T for c=1 cores, so each core's own key chunk j
always occupies columns [512j, 512j+256). The q projection then produces
queries in the same swapped order; window masks (per-core data) account for
it, and the host swaps the output columns back.

Each core produces PARTIAL attention sums: num^T[H, T] = sum_k exp(s)·v and
den[1, T] = sum_k exp(s); the host combines halves and normalizes:
out = ((num0+num1)/(den0+den1))^T + bv.

Per-core pipeline (fp16 matmul inputs, fp32 PSUM accumulation):
  e-major projections (W stationary per 128-chunk of E): kT,vT [H,1024 own],
  qT [H,2048]; vT --PE transpose--> v natural [keys,H] per 128-key slot;
  per group g: scores^T[keys,512q] = kT_s^T @ qT_g (slots 0..2g+1), window
  masks on the last two slots, exp (ACT, fp16), AV with v stationary
  accumulating num^T, denominator via DVE slot-sums + one ones-row matmul.
Softmax skips max-subtraction: scores ~ N(0,1), exp is safe in fp16 range.
"""

import math

import numpy as np

import concourse.bass as bass
import concourse.tile as tile
from concourse import bacc, mybir
from concourse.bass_utils import run_bass_kernel_spmd
from concourse.masks import make_identity

B, T, E, H = 4, 2048, 1024, 128
NE = E // 128           # 8 contraction chunks
NS = 8                  # own key slots (128 keys each) per core
BIG = 1.0e30
N_WARM = 18             # PE p-state warmup matmuls while x streams in

F32 = mybir.dt.float32
F16 = mybir.dt.float16

_CACHE: dict = {}


def _build(debug=False):
    nc = bacc.Bacc(None, target_bir_lowering=False)
    xt = nc.dram_tensor("xt", [E, T], F16, kind="ExternalInput")
    wcat = nc.dram_tensor("wcat", [128, 3 * E], F16, kind="ExternalInput")
    bq = nc.dram_tensor("bq", [H, 1], F32, kind="ExternalInput")
    bk = nc.dram_tensor("bk", [H, 1], F32, kind="ExternalInput")
    msk = nc.dram_tensor("msk", [128, 1024], F32, kind="ExternalInput")
    y_num = nc.dram_tensor("y_num", [H, T], F16, kind="ExternalOutput")
    y_den = nc.dram_tensor("y_den", [1, T], F32, kind="ExternalOutput")
    if debug:
        dbg_q = nc.dram_tensor("dbg_q", [H, T], F16, kind="ExternalOutput")
        dbg_k = nc.dram_tensor("dbg_k", [H, NS * 128], F16, kind="ExternalOutput")
        dbg_v = nc.dram_tensor("dbg_v", [H, NS * 128], F16, kind="ExternalOutput")
        dbg_vn = nc.dram_tensor("dbg_vn", [128, NS * 128], F16, kind="ExternalOutput")

    with tile.TileContext(nc) as tc:
        with (
            tc.tile_pool(name="xs", bufs=1) as xs_pool,
            tc.tile_pool(name="wsb", bufs=1) as w_pool,
            tc.tile_pool(name="small", bufs=1) as sm_pool,
            tc.tile_pool(name="qk", bufs=1) as qk_pool,
            tc.tile_pool(name="ex", bufs=10) as ex_pool,
        ):
            # ---- DMA in (ACT HWDGE queue, in consumption order)
            wsb = w_pool.tile([128, 3 * E], F16)
            nc.scalar.dma_start(wsb[:], wcat[:])
            xs = xs_pool.tile([128, NE * T], F16, name="xs", tag="xs")
            for e in range(NE):
                nc.scalar.dma_start(
                    xs[:, e * T:(e + 1) * T], xt[e * 128:(e + 1) * 128, :])
            # small inputs on the SP HWDGE queue
            bq_sb = sm_pool.tile([128, 1], F32, tag="bq")
            bk_sb = sm_pool.tile([128, 1], F32, tag="bk")
            nc.sync.dma_start(bq_sb[:], bq[:])
            nc.sync.dma_start(bk_sb[:], bk[:])
            mask_sb = sm_pool.tile([128, 1024], F32, tag="msk")
            nc.sync.dma_start(mask_sb[:], msk[:])

            ident = sm_pool.tile([128, 128], F16, tag="ident")
            make_identity(nc, ident[:])
            ones = sm_pool.tile([128, 1], F16, tag="ones")
            nc.vector.memset(ones[:], 1.0)
            scratch = sm_pool.tile([128, 512], F16, tag="scratch")
            nc.vector.memset(scratch[:], 0.0)

            kT = qk_pool.tile([128, NS * 128], F16, tag="kT")
            vT = qk_pool.tile([128, NS * 128], F16, tag="vT")
            vnat = qk_pool.tile([128, NS * 128], F16, tag="vnat")
            qT = qk_pool.tile([128, T], F16, tag="qT")

            # ---- projections, e-major (one pass; 8 PSUM banks)
            with (
                tc.tile_pool(name="qps", bufs=1, space="PSUM") as qps_pool,
                tc.tile_pool(name="kvps", bufs=1, space="PSUM") as kvps,
            ):
                qp = [qps_pool.tile([128, 512], F32, name=f"qp{g}", tag=f"qp{g}")
                      for g in range(4)]
                kps = [kvps.tile([128, 512], F32, name=f"kps{p}", tag=f"kps{p}")
                       for p in range(2)]
                vps = [kvps.tile([128, 512], F32, name=f"vps{p}", tag=f"vps{p}")
                       for p in range(2)]

                # warm the PE p-state while x streams in (qp0 is cleared by
                # the first real accumulation's start=True)
                for i in range(N_WARM):
                    nc.tensor.matmul(qp[i % 2][:], ident[:], scratch[:],
                                     start=True, stop=True)

                def own(e, p):
                    # own-key cols of chunk pair p: [1024p+{0:256, 512:768}]
                    base = e * T + 1024 * p
                    return (xs[:, base: base + 1024]
                            .rearrange("a (b c) -> a b c", b=2)[:, :, 0:256])

                for e in range(NE):
                    st, fin = (e == 0), (e == NE - 1)
                    for p in range(2):
                        nc.tensor.matmul(
                            kps[p][:], wsb[:, e * 128:(e + 1) * 128],
                            own(e, p), start=st, stop=fin)
                    for p in range(2):
                        nc.tensor.matmul(
                            vps[p][:], wsb[:, E + e * 128: E + (e + 1) * 128],
                            own(e, p), start=st, stop=fin)
                    for g in range(4):
                        nc.tensor.matmul(
                            qp[g][:],
                            wsb[:, 2 * E + e * 128: 2 * E + (e + 1) * 128],
                            xs[:, e * T + g * 512: e * T + (g + 1) * 512],
                            start=st, stop=fin)
                for i in range(2):
                    nc.scalar.activation(
                        kT[:, i * 512:(i + 1) * 512], kps[i][:],
                        mybir.ActivationFunctionType.Identity, bias=bk_sb[:])
                    nc.vector.tensor_copy(vT[:, i * 512:(i + 1) * 512],
                                          vps[i][:])
                for g in range(4):
                    nc.scalar.activation(
                        qT[:, g * 512:(g + 1) * 512], qp[g][:],
                        mybir.ActivationFunctionType.Identity, bias=bq_sb[:])

            if debug:
                nc.sync.dma_start(dbg_q[:], qT[:])
                nc.sync.dma_start(dbg_k[:], kT[:])
                nc.sync.dma_start(dbg_v[:], vT[:])

            # ---- v transposes + attention
            with (
                tc.tile_pool(name="tps", bufs=2, space="PSUM") as tps,
                tc.tile_pool(name="sps", bufs=3, space="PSUM") as sps,
                tc.tile_pool(name="aps", bufs=2, space="PSUM") as aps,
                tc.tile_pool(name="dps", bufs=1, space="PSUM") as dps,
            ):
                for s in range(NS):
                    tp = tps.tile([128, 128], F16)
                    nc.tensor.transpose(
                        tp[:], vT[:, s * 128:(s + 1) * 128], ident[:])
                    nc.vector.tensor_copy(vnat[:, s * 128:(s + 1) * 128], tp[:])
                if debug:
                    nc.sync.dma_start(dbg_vn[:], vnat[:])

                for g in range(4):
                    L = 2 * g + 2
                    av = aps.tile([128, 512], F32, name=f"av{g}", tag="av")
                    den = dps.tile([1, 512], F32, name=f"den{g}", tag="den")
                    sum_ex = ex_pool.tile([128, 512], F16, name=f"sume{g}",
                                          tag="sumex")
                    exs = []
                    for s in range(L):
                        sp = sps.tile([128, 512], F32)
                        nc.tensor.matmul(
                            sp[:], kT[:, s * 128:(s + 1) * 128],
                            qT[:, g * 512:(g + 1) * 512], start=True, stop=True)
                        if s >= L - 2:
                            j = s - (L - 2)
                            nc.vector.tensor_add(
                                sp[:], sp[:], mask_sb[:, j * 512:(j + 1) * 512])
                        ex = ex_pool.tile([128, 512], F16)
                        nc.scalar.activation(
                            ex[:], sp[:], mybir.ActivationFunctionType.Exp)
                        exs.append(ex)
                        nc.tensor.matmul(
                            av[:], vnat[:, s * 128:(s + 1) * 128], ex[:],
                            start=(s == 0), stop=(s == L - 1))
                        if s == 1:
                            nc.vector.tensor_add(sum_ex[:], exs[0][:], exs[1][:])
                        elif s > 1:
                            nc.vector.tensor_add(sum_ex[:], sum_ex[:], ex[:])
                    nc.tensor.matmul(den[:], ones[:], sum_ex[:], start=True,
                                     stop=True)
                    av_sb = ex_pool.tile([128, 512], F16, name=f"avsb{g}",
                                         tag="avsb")
                    nc.vector.tensor_copy(av_sb[:], av[:])
                    den_sb = ex_pool.tile([1, 512], F32, name=f"densb{g}",
                                          tag="densb")
                    nc.vector.tensor_copy(den_sb[:], den[:])
                    nc.sync.dma_start(y_num[:, g * 512:(g + 1) * 512], av_sb[:])
                    nc.sync.dma_start(y_den[:, g * 512:(g + 1) * 512], den_sb[:])
    nc.compile()
    return nc


def _masks(c: int) -> np.ndarray:
    p = np.arange(128)[:, None]  # key within block (partition)
    q = np.arange(128)[None, :]  # query within block (free)
    tri = np.where(p <= q, 0.0, -BIG).astype(np.float32)
    z = np.zeros((128, 128), np.float32)
    inv = np.full((128, 128), -BIG, np.float32)
    pi = (0, 1, 2, 3) if c == 0 else (2, 3, 0, 1)  # natural q block at quarter
    parts = []
    for r in (2 * c, 2 * c + 1):  # own window key blocks 4g+r
        quarters = [tri if r == pi[cq] else (z if r < pi[cq] else inv)
                    for cq in range(4)]
        parts.append(np.concatenate(quarters, axis=1))
    return np.ascontiguousarray(np.concatenate(parts, axis=1))


def _half_swap(a: np.ndarray) -> np.ndarray:
    # swap the two 256-column halves of every 512-column quarter
    n = a.shape[-1] // 512
    return np.ascontiguousarray(
        a.reshape(*a.shape[:-1], n, 2, 256)[..., ::-1, :].reshape(*a.shape))


def kernel(x, Wq, bq, Wk, bk, Wv, bv):
    x = np.asarray(x, dtype=np.float32)
    Wq = np.asarray(Wq, dtype=np.float32)
    Wk = np.asarray(Wk, dtype=np.float32)
    Wv = np.asarray(Wv, dtype=np.float32)
    bq = np.asarray(bq, dtype=np.float32)
    bk = np.asarray(bk, dtype=np.float32)
    bv = np.asarray(bv, dtype=np.float32)

    if "nc" not in _CACHE:
        _CACHE["nc"] = _build()
    nc = _CACHE["nc"]

    scale = 1.0 / math.sqrt(H)
    # wcat[p, proj*E + e*128 + h] = W[e*128 + p, h], proj order (k, v, q)
    wcat = np.empty((128, 3 * E), dtype=np.float16)
    for pi_, w in enumerate((Wk, Wv, Wq * scale)):
        wcat[:, pi_ * E:(pi_ + 1) * E] = (
            w.astype(np.float16).reshape(NE, 128, H).transpose(1, 0, 2)
            .reshape(128, E))
    bq_s = np.ascontiguousarray((bq * scale).reshape(H, 1))
    bk_r = np.ascontiguousarray(bk.reshape(H, 1))
    masks = {0: _masks(0), 1: _masks(1)}

    xt = np.ascontiguousarray(x.transpose(0, 2, 1).astype(np.float16))  # [B,E,T]
    xt_sw = {0: xt, 1: None}
    in_maps = []
    for core in range(8):
        b, c = divmod(core, 2)
        xtc = xt[b] if c == 0 else _half_swap(xt[b])
        in_maps.append({
            "xt": xtc, "wcat": wcat,
            "bq": bq_s, "bk": bk_r, "msk": masks[c],
        })

    res = run_bass_kernel_spmd(nc, in_maps, core_ids=list(range(8)))
    out = np.empty((B, T, H), dtype=np.float32)
    for b in range(B):
        r0, r1 = res.results[2 * b], res.results[2 * b + 1]
        num = (r0["y_num"].astype(np.float32)
               + _half_swap(r1["y_num"]).astype(np.float32))   # [H, T]
        den = r0["y_den"] + _half_swap(r1["y_den"])            # [1, T]
        out[b] = (num / den).T
    out += bv  # sum_j softmax_ij = 1, so +bv commutes with attention
    return out
